# revision 45
# baseline (speedup 1.0000x reference)
"""Multi-head causal attention (B=4, C=2048, E=1024, H=16, D=64) on 8 TRN2 cores.

Sharding: batch x head-group (4 x 2). Core c handles batch c//2 and heads
(c%2)*8 .. (c%2)*8+8.  Each core computes a partial output

    Y_c = Attn(x_b; heads hg) @ W_o[hg rows]        (shape [C, E])

and the host sums the two partials per batch (row-split W_o all-reduce done
host-side since outputs are gathered anyway).

v4 structure (one software-pipelined loop):
  * all matmul operands bf16 (fp32 PSUM accumulation) - fp32r was
    power-throttled on HW; bf16 also halves LDWEIGHTS and DMA.
  * BIR post-pass merges the tile_legalize-presplit Ldweights back into
    self-loading Matmults so walrus --enable-ldw-opt=true (flipped by a
    run_command patch) can overlap weight loads with matmul streaming.
  * projections of q-slice j+1 and the output projection of earlier
    slices are interleaved as PE "filler" work BETWEEN a group's S^T
    matmuls and its P@V (the PE queue is in-order; this placement is
    what actually covers the softmax-exp latency P@V waits on).
  * causal diagonal blocks restricted to the valid column range for the
    S^T matmul, the exp, and the P@V accumulation; S PSUM tiles are one
    bank per (kt, half) and exp runs per kt, so PSUM banks recycle at
    the finest granularity the 8-bank budget allows.
  * softmax denominator: the V tiles carry a ones column so the
    denominator rides in PSUM row 64 of the P@V output; per (head, slice)
    it is staged to SBUF, DMA-reshaped [1,512]->[64,8] (DVE reciprocal
    cost is per-column, so this is ~20x cheaper than reciprocal on the
    row), reciprocal'd in fp32r, DMA'd back, broadcast across partitions
    by a K=1 fp32r ones-matmul at tile_position (64,0), and applied by a
    DVE multiply (deferred one head-pair so the chain never stalls PE).
  * output returned in bf16 (host upcasts); tail out-projections
    alternate PSUM pools and spread their y DMAs over the gpsimd/sync/
    scalar queues so the drain isn't serialized on one queue.
"""

import sys

if "/opt/trn_rl_repo" not in sys.path:
    sys.path.insert(0, "/opt/trn_rl_repo")

import math

import numpy as np

B, C, E, H, D = 4, 2048, 1024, 16, 64
NCORES = 8
P = 128
CS = 512  # q-slice width


def build_module(C=C, E=E, HL=H // 2, D=D, n_devices=NCORES):
    """Build the SPMD Bass module for one core's shard."""
    from contextlib import ExitStack

    import concourse.bass as bass
    import concourse.mybir as mybir
    import concourse.tile as tile

    F32 = mybir.dt.float32
    FR = mybir.dt.float32r
    BF = mybir.dt.bfloat16
    Exp = mybir.ActivationFunctionType.Exp
    MUL = mybir.AluOpType.mult
    DIV = mybir.AluOpType.divide
    ISGE = mybir.AluOpType.is_ge

    ET = E // P          # e-tiles
    JT = HL * D // P     # j-tiles (head pairs)
    NJ = C // CS         # q-slices
    CT = C // P          # c-tiles
    KPJ = CS // P        # kk-tiles per q-slice (4)
    scale = 1.0 / math.sqrt(D)

    nc = bass.Bass(
        "TRN2", target_bir_lowering=False, debug=False, num_devices=n_devices
    )

    xT = nc.dram_tensor("xT", [P, ET, C], BF, kind="ExternalInput").ap()
    wq_d = nc.dram_tensor("wq", [P, ET, HL * D], BF, kind="ExternalInput").ap()
    wk_d = nc.dram_tensor("wk", [P, ET, HL * D], BF, kind="ExternalInput").ap()
    wv_d = nc.dram_tensor("wv", [P, ET, HL * D], BF, kind="ExternalInput").ap()
    wo_d = nc.dram_tensor("wo", [P, JT, E], BF, kind="ExternalInput").ap()
    y_d = nc.dram_tensor("y", [CT, P, E], BF, kind="ExternalOutput").ap()

    with tile.TileContext(nc) as tc:
        with ExitStack() as ctx:
            pA = ctx.enter_context(tc.tile_pool(name="pA", bufs=1))
            psS = ctx.enter_context(tc.tile_pool(name="psS", bufs=4, space="PSUM"))
            psPV = ctx.enter_context(tc.tile_pool(name="psPV", bufs=2, space="PSUM"))
            psMM = ctx.enter_context(tc.tile_pool(name="psMM", bufs=2, space="PSUM"))
            pX = ctx.enter_context(tc.tile_pool(name="pX", bufs=2))
            pE = ctx.enter_context(tc.tile_pool(name="pE", bufs=10))
            pT = ctx.enter_context(tc.tile_pool(name="pT", bufs=4))
            pH = ctx.enter_context(tc.tile_pool(name="pH", bufs=10))
            pD = ctx.enter_context(tc.tile_pool(name="pD", bufs=2))
            pF = ctx.enter_context(tc.tile_pool(name="pF", bufs=2))
            pR = ctx.enter_context(tc.tile_pool(name="pR", bufs=2))

            qt = pA.tile([P, JT, C], BF, tag="qt")
            kt = pA.tile([P, JT, C], BF, tag="kt")
            v = pA.tile([P, CT, HL, D + 1], BF, tag="v")
            hdt = pA.tile([P, JT, C], BF, tag="hdt")
            ones = pA.tile([P, 64], FR, tag="ones")
            wq = pA.tile([P, ET, HL * D], BF, tag="wq")
            wk = pA.tile([P, ET, HL * D], BF, tag="wk")
            wv = pA.tile([P, ET, HL * D], BF, tag="wv")
            wo = pA.tile([P, JT, E], BF, tag="wo")

            xts = {}

            def load_x(cs, split=False):
                xt = pX.tile([P, ET, CS], BF, tag="xt")
                csl = slice(cs * CS, (cs + 1) * CS)
                if split:
                    h = ET // 2
                    nc.sync.dma_start(xt[:, 0:h, :], xT[:, 0:h, csl])
                    nc.scalar.dma_start(xt[:, h:ET, :], xT[:, h:ET, csl])
                else:
                    nc.sync.dma_start(xt[:], xT[:, :, csl])
                xts[cs] = xt

            # the first matmul chain consumes (wq[et], x0[et]) pairs in
            # order: issue them as alternating per-et transfers on the two
            # HWDGE queues so the chain starts as soon as et=0 lands
            xt0 = pX.tile([P, ET, CS], BF, tag="xt")
            xts[0] = xt0
            for et in range(ET):
                qa, qb = (nc.sync, nc.scalar) if et % 2 == 0 else (
                    nc.scalar,
                    nc.sync,
                )
                qa.dma_start(wq[:, et, :], wq_d[:, et, :])
                qb.dma_start(xt0[:, et, :], xT[:, et, 0:CS])
            # wk/wv/wo stream on the vector+gpsimd queues: sync+scalar are
            # saturated feeding the wq/x0 interleave the first Q units
            # consume, and DVE/Pool are idle this early
            h = ET // 2
            nc.gpsimd.dma_start(wk[:, 0:h, :], wk_d[:, 0:h, :])
            nc.gpsimd.dma_start(wk[:, h:ET, :], wk_d[:, h:ET, :])
            nc.gpsimd.dma_start(wv[:, 0:h, :], wv_d[:, 0:h, :])
            nc.gpsimd.dma_start(wv[:, h:ET, :], wv_d[:, h:ET, :])
            nc.gpsimd.dma_start(wo[:], wo_d)
            nc.vector.memset(ones[:].bitcast(F32), 1.0)
            nc.vector.memset(v[:, :, :, D : D + 1], 1.0)

            def proj_units(cs):
                """Projection work for q-slice cs as a list of callables,
                each ~8 matmuls + 1 psum evict."""
                csl = slice(cs * CS, (cs + 1) * CS)

                def qk_unit(w_sb, out_t, jt):
                    def run():
                        xt = xts[cs]
                        ps = psMM.tile([P, CS], F32, tag="mm")
                        for et in range(ET):
                            nc.tensor.matmul(
                                ps[:],
                                w_sb[:, et, jt * P : (jt + 1) * P],
                                xt[:, et, :],
                                start=(et == 0),
                                stop=(et == ET - 1),
                            )
                        nc.vector.tensor_copy(out_t[:, jt, csl], ps[:])

                    return run

                def v_unit(c4):
                    def run():
                        xt = xts[cs]
                        ct = cs * KPJ + c4
                        ps = psMM.tile([P, HL, D], F32, tag="mm")
                        for et in range(ET):
                            nc.tensor.matmul(
                                ps[:],
                                xt[:, et, c4 * P : (c4 + 1) * P],
                                wv[:, et, :],
                                start=(et == 0),
                                stop=(et == ET - 1),
                            )
                        nc.vector.tensor_copy(v[:, ct, :, 0:D], ps[:])

                    return run

                units = []
                if cs == 0:
                    # slice 0 runs at t=0: Q units only need wq + x0, which
                    # stream in first; K/V weights land while they run
                    for jt in range(JT):
                        units.append(qk_unit(wq, qt, jt))
                    for jt in range(JT):
                        units.append(qk_unit(wk, kt, jt))
                else:
                    for jt in range(JT):
                        units.append(qk_unit(wq, qt, jt))
                        units.append(qk_unit(wk, kt, jt))
                for c4 in range(KPJ):
                    units.append(v_unit(c4))
                return units

            def outproj_units(jj, alt=False):
                """Output projection for the c-tiles of q-slice jj.  With
                alt=True (tail drain, attention done) alternate psMM/psS
                so the PE doesn't wait on single-pool PSUM rotation."""
                FS = min(CS, E)
                units = []
                for c4 in range(KPJ):
                    for fs in range(E // FS):

                        def run(
                            ct=jj * KPJ + c4,
                            fsl=slice(fs * FS, (fs + 1) * FS),
                            pool=(psS if alt and (c4 % 2) else psMM),
                            dq=(
                                [nc.gpsimd, nc.sync, nc.scalar][
                                    (c4 * (E // FS) + fs) % 3
                                ]
                                if alt
                                else nc.gpsimd
                            ),
                        ):
                            ps = pool.tile([P, FS], F32, tag="mm" if pool is psMM else "s")
                            for jt in range(JT):
                                nc.tensor.matmul(
                                    ps[:],
                                    hdt[:, jt, ct * P : (ct + 1) * P],
                                    wo[:, jt, fsl],
                                    start=(jt == 0),
                                    stop=(jt == JT - 1),
                                )
                            ysb = pT.tile([P, FS], BF, tag="ysb")
                            nc.vector.tensor_copy(ysb[:], ps[:])
                            dq.dma_start(y_d[ct, :, fsl], ysb[:])

                        units.append(run)
                return units

            for u in proj_units(0):
                u()

            for j in range(NJ):
                jsl = slice(j * CS, (j + 1) * CS)
                nkt = (j + 1) * KPJ  # kk-tiles needed (causal)
                fillers = []
                if j + 1 < NJ:
                    load_x(j + 1)
                    fillers += proj_units(j + 1)
                # out-projections run one slice later than they become
                # ready (the last normalize of slice jj lands while slice
                # jj+1's first groups run, so jj's outproj can only be
                # dispatched from jj+2); spread them 2/1 so slice 3's
                # DVE/DMA load doesn't spike
                if j == 2:
                    fillers += outproj_units(0) + outproj_units(1)[:4]
                elif j == 3:
                    fillers += outproj_units(1)[4:] + outproj_units(2)
                ngrps = JT * (j + 1)
                gdone = fi = 0
                norms = []

                def emit_norm(g, dt, hds, jsl=jsl):
                    bts = {}

                    def mk_bcast(half):
                        def run():
                            bct = psS.tile([P, CS], F32, tag="s", name="bc")
                            bc = bct[0:64, :]
                            nc.tensor.matmul(
                                bc,
                                ones[64:65, :],
                                dt[D : D + 1, half, :],
                                start=True,
                                stop=True,
                                tile_position=(64, 0),
                            )
                            bts[half] = bc

                        return run

                    def mk_mul(half):
                        def run():
                            hd = hds[half]
                            if half == 0:
                                nc.vector.tensor_tensor(
                                    hdt[0:64, g, jsl], hd[:], bts[half], MUL
                                )
                            else:
                                tmp = pT.tile([64, CS], BF, tag="tmp")
                                nc.vector.tensor_tensor(
                                    tmp[:], hd[:], bts[half], MUL
                                )
                                nc.sync.dma_start(
                                    hdt[64:128, g, jsl], tmp[:]
                                )

                        return run

                    for half in (1, 0):
                        norms.append(mk_bcast(half))
                    for half in (1, 0):
                        norms.append(mk_mul(half))

                pending = None
                for g in range(JT):
                    pv_ps = [
                        psPV.tile([D + 1, CS], F32, tag="pv", name=f"pv{h}")
                        for h in range(2)
                    ]
                    # kk-tiles in groups of 4 (two 2-kt psum chunks) so the
                    # S^T matmuls and the PV accumulation run as longer
                    # back-to-back chains on the PE
                    for grp in range((nkt + 3) // 4):
                        group = []  # (kts, ws, s_ps, e_sb) per 2-kt chunk
                        for ck in (2 * grp, 2 * grp + 1):
                            kts = [k for k in (2 * ck, 2 * ck + 1) if k < nkt]
                            if not kts:
                                continue
                            ws = [max(0, k * P - j * CS) for k in kts]
                            # one PSUM bank per (kt, half): exp releases
                            # banks at kt granularity, so the next chunk's
                            # S matmuls start half a chunk earlier
                            s_ps = [
                                [
                                    psS.tile(
                                        [P, CS], F32, tag="s", name=f"s{h}{i}"
                                    )
                                    for h in range(2)
                                ]
                                for i in range(len(kts))
                            ]
                            e_sb = [
                                pE.tile([P, 2, CS], BF, tag="e", name=f"e{h}")
                                for h in range(2)
                            ]
                            group.append((kts, ws, s_ps, e_sb))
                            for i, kkt in enumerate(kts):
                                w = ws[i]
                                ksl = slice(kkt * P, (kkt + 1) * P)
                                qsl = slice(j * CS + w, (j + 1) * CS)
                                for half, base in ((0, 0), (1, 64)):
                                    nc.tensor.matmul(
                                        s_ps[i][half][:, w:CS],
                                        kt[base : base + 64, g, ksl],
                                        qt[base : base + 64, g, qsl],
                                        start=True,
                                        stop=True,
                                        tile_position=(base, 0),
                                    )
                        for kts, ws, s_ps, e_sb in group:
                            for i in range(len(kts)):
                                w = ws[i]
                                for half in range(2):
                                    nc.scalar.activation(
                                        e_sb[half][:, i, w:CS],
                                        s_ps[i][half][:, w:CS],
                                        Exp,
                                        scale=scale,
                                    )
                            for i, kkt in enumerate(kts):
                                w = ws[i]
                                if 0 <= kkt * P - j * CS < CS:
                                    for half in range(2):
                                        blk = e_sb[half][:, i, w : w + P]
                                        nc.gpsimd.affine_select(
                                            blk,
                                            blk,
                                            pattern=[[1, P]],
                                            compare_op=ISGE,
                                            fill=0.0,
                                            base=0,
                                            channel_multiplier=-1,
                                        )
                        gdone += 1
                        # filler PE work goes BETWEEN this group's S matmuls
                        # and its P@V: the PE queue is in-order, so this is
                        # what actually covers the exp(ACT) latency the P@V
                        # matmuls wait on.  (hold a few units back for the
                        # tail normalize.)
                        want = len(fillers) * gdone // ngrps
                        if grp == (nkt + 3) // 4 - 1:
                            want = max(want, fi + 2)
                        want = min(want, max(0, len(fillers) - 3))
                        while fi < want:
                            fillers[fi]()
                            fi += 1
                        for half in range(2):
                            h = 2 * g + half
                            for kts, ws, s_ps, e_sb in group:
                                for i, kkt in enumerate(kts):
                                    w = ws[i]
                                    nc.tensor.matmul(
                                        pv_ps[half][:, w:CS],
                                        v[:, kkt, h, :],
                                        e_sb[half][:, i, w:CS],
                                        start=(kkt == 0),
                                        stop=(kkt == nkt - 1),
                                    )

                    # evict PV (bf16); the raw denominators ride in PSUM row
                    # 64 (ones column of V) — reciprocal each in place on
                    # that single row (fast approx, ~18 bits), then gpsimd
                    # broadcasts it across partitions for the DVE multiply.
                    hds = []
                    for half in range(2):
                        hd = pH.tile([D, CS], BF, tag="hd", name=f"hd{half}")
                        nc.vector.tensor_copy(hd[:], pv_ps[half][0:D, :])
                        hds.append(hd)
                    # DVE reciprocal cost is per-column, so reshape the
                    # [1,512] denominator row into [64,8] via DMA, take the
                    # reciprocal over 8 columns (~30x cheaper than on the
                    # row), and DMA back for the fp32r broadcast matmul
                    st = pD.tile([D + 1, 2, CS], F32, tag="st")
                    dt = pF.tile([D + 1, 2, CS], FR, tag="dt")
                    dr = pR.tile([64, 2, 8], FR, tag="dr")
                    for half in range(2):
                        nc.vector.tensor_copy(
                            st[D : D + 1, half, :],
                            pv_ps[half][D : D + 1, :],
                        )
                        nc.sync.dma_start(
                            dr[:, half, :].bitcast(F32),
                            st[D : D + 1, half, :],
                        )
                    with nc.allow_low_precision(
                        reason="fp32r reciprocal feeds fp32r matmul"
                    ):
                        for half in range(2):
                            nc.vector.reciprocal(
                                dr[:, half, :], dr[:, half, :]
                            )
                    for half in range(2):
                        nc.sync.dma_start(
                            dt[D : D + 1, half, :],
                            dr[:, half, :],
                        )
                    # previous g's broadcast+normalize first: its psS slot
                    # is exp-released and its DVE multiplies must not queue
                    # behind this g's hd evictions
                    if pending is not None:
                        for u in pending:
                            u()
                    emit_norm(g, dt, hds)
                    pending, norms = norms, []

                while fi < len(fillers):
                    fillers[fi]()
                    fi += 1
                for u in pending:
                    u()

            for u in outproj_units(NJ - 1, alt=True):
                u()
    return nc


def _merge_ldweights_json(bir_json_bytes):
    """tile_legalize pre-splits every non-f32 matmul into a standalone
    InstLdweights + non-self-loading InstMatmult, but walrus's
    --enable-ldw-opt=true rejects pre-split Ldweights ("not compatible
    with LDW optimization").  Merge each pair back into a self-loading
    Matmult (ldweights=True, waits unioned) so walrus can apply its own
    LDWEIGHTS overlap optimization."""
    import json

    d = json.loads(bir_json_bytes)
    n = 0
    for fn in d["functions"]:
        for blk in fn["blocks"]:
            insts = blk["instructions"]
            out = []
            pending = None  # pending Ldweights awaiting its Matmult
            for inst in insts:
                if inst.get("opcode") == "Ldweights":
                    assert pending is None, "two Ldweights without a Matmult"
                    pending = inst
                    continue
                if pending is not None and inst.get("engine") == "PE":
                    assert inst.get("opcode") == "Matmult", (
                        f"Ldweights followed by PE {inst.get('opcode')}"
                    )
                    w_ld = pending["ins"][0]
                    w_mm = inst["ins"][1]
                    assert (
                        w_ld["memref"] == w_mm["memref"]
                        and w_ld["offset"] == w_mm["offset"]
                    ), "Ldweights/Matmult weight AP mismatch"
                    inst["ldweights"] = True
                    lsi = pending.get("sync_info") or {}
                    msi = inst.get("sync_info") or {}
                    waits = (lsi.get("on_wait") or []) + (msi.get("on_wait") or [])
                    updates = (lsi.get("on_update") or []) + (
                        msi.get("on_update") or []
                    )
                    if waits or updates:
                        inst["sync_info"] = {"on_wait": waits, "on_update": updates}
                    pending = None
                    n += 1
                out.append(inst)
            assert pending is None, "trailing Ldweights without a Matmult"
            blk["instructions"] = out
    return json.dumps(d).encode()


def _patch_ldw_opt():
    """Flip walrus to --enable-ldw-opt=true (the merged self-loading
    matmuls above are compatible; the default pre-split ones are not)."""
    import concourse.bass_utils as bu

    if getattr(bu, "_ldw_opt_patched", False):
        return
    bu._ldw_opt_patched = True
    orig = bu.run_command

    def patched(argv, **kw):
        argv = [
            a.replace("--enable-ldw-opt=false", "--enable-ldw-opt=true")
            for a in argv
        ]
        return orig(argv, **kw)

    bu.run_command = patched


def _split_waits_json(bir_json_bytes):
    """TRN2 TPB instructions have one sync-wait slot and this walrus build
    refuses to split multi-wait instructions, so hoist all but the last wait
    onto preceding wait-only EventSemaphore instructions (same engine,
    executed in order -> semantically identical)."""
    import json

    d = json.loads(bir_json_bytes)
    n = 0
    for fn in d["functions"]:
        for blk in fn["blocks"]:
            out = []
            for inst in blk["instructions"]:
                si = inst.get("sync_info")
                waits = (si or {}).get("on_wait") or []
                if len(waits) > 1:
                    for w in waits[:-1]:
                        n += 1
                        out.append(
                            {
                                "debug": inst.get("debug", 0),
                                "engine": inst["engine"],
                                "ins": [],
                                "name": f"wsplit-{n}",
                                "opcode": "EventSemaphore",
                                "outs": [],
                                "sync_info": {"on_update": [], "on_wait": [w]},
                            }
                        )
                    si["on_wait"] = [waits[-1]]
                out.append(inst)
            blk["instructions"] = out
    return json.dumps(d).encode()


def _striped(a, p=P):
    """[K, N] with K = kt*p + i  ->  contiguous [p, K//p, N]."""
    k, n = a.shape
    return np.ascontiguousarray(a.reshape(k // p, p, n).transpose(1, 0, 2))


def _bf16(a):
    import ml_dtypes

    return a.astype(ml_dtypes.bfloat16)


def prep_core_inputs(x_b, wq_s, wk_s, wv_s, wo_s):
    """Host-side layout prep for one core. x_b [C,E], w*_s column/row slices."""
    return {
        "xT": _bf16(_striped(np.ascontiguousarray(x_b.T))),
        "wq": _bf16(_striped(wq_s)),
        "wk": _bf16(_striped(wk_s)),
        "wv": _bf16(_striped(wv_s)),
        "wo": _bf16(_striped(wo_s)),
    }


_module_cache = {}


def kernel(x, W_q, W_k, W_v, W_o):
    from concourse.bass_utils import run_bass_kernel_spmd

    x = np.asarray(x, dtype=np.float32)
    W_q = np.asarray(W_q, dtype=np.float32)
    W_k = np.asarray(W_k, dtype=np.float32)
    W_v = np.asarray(W_v, dtype=np.float32)
    W_o = np.asarray(W_o, dtype=np.float32)

    HD2 = H * D // 2  # columns per head-group (512)
    in_maps = []
    for core in range(NCORES):
        b, hg = core // 2, core % 2
        cols = slice(hg * HD2, (hg + 1) * HD2)
        in_maps.append(
            prep_core_inputs(
                x[b], W_q[:, cols], W_k[:, cols], W_v[:, cols], W_o[cols, :]
            )
        )

    if "nc" not in _module_cache:
        _patch_ldw_opt()
        nc = build_module()
        fixed = _split_waits_json(_merge_ldweights_json(nc.to_json_bytes()))
        nc.to_json_bytes = lambda: fixed
        _module_cache["nc"] = nc
    nc = _module_cache["nc"]

    res = run_bass_kernel_spmd(nc, in_maps, core_ids=list(range(NCORES)))
    _module_cache["last_res"] = res
    out = np.empty((B, C, E), dtype=np.float32)
    for b in range(B):
        ya = res.results[2 * b]["y"].astype(np.float32).reshape(C, E)
        yb = res.results[2 * b + 1]["y"].astype(np.float32).reshape(C, E)
        out[b] = ya + yb
    return out


if __name__ == "__main__":
    rng = np.random.default_rng(0)
    ins = {
        "x": rng.standard_normal((B, C, E), dtype=np.float32),
        "W_q": rng.standard_normal((E, H * D), dtype=np.float32) * 0.02,
        "W_k": rng.standard_normal((E, H * D), dtype=np.float32) * 0.02,
        "W_v": rng.standard_normal((E, H * D), dtype=np.float32) * 0.02,
        "W_o": rng.standard_normal((H * D, E), dtype=np.float32) * 0.02,
    }
    out = kernel(**ins)
    print("kernel ran, out shape", out.shape, "mean", out.mean())



# revision 47
# speedup vs baseline: 1.0256x; 1.0256x over previous
"""Multi-head causal attention (B=4, C=2048, E=1024, H=16, D=64) on 8 TRN2 cores.

Sharding: batch x head-group (4 x 2). Core c handles batch c//2 and heads
(c%2)*8 .. (c%2)*8+8.  Each core computes a partial output

    Y_c = Attn(x_b; heads hg) @ W_o[hg rows]        (shape [C, E])

and the host sums the two partials per batch (row-split W_o all-reduce done
host-side since outputs are gathered anyway).

v4 structure (one software-pipelined loop):
  * all matmul operands bf16 (fp32 PSUM accumulation) - fp32r was
    power-throttled on HW; bf16 also halves LDWEIGHTS and DMA.
  * BIR post-pass merges the tile_legalize-presplit Ldweights back into
    self-loading Matmults so walrus --enable-ldw-opt=true (flipped by a
    run_command patch) can overlap weight loads with matmul streaming.
  * projections of q-slice j+1 and the output projection of earlier
    slices are interleaved as PE "filler" work BETWEEN a group's S^T
    matmuls and its P@V (the PE queue is in-order; this placement is
    what actually covers the softmax-exp latency P@V waits on).
  * causal diagonal blocks restricted to the valid column range for the
    S^T matmul, the exp, and the P@V accumulation; S PSUM tiles are one
    bank per (kt, half) and exp runs per kt, so PSUM banks recycle at
    the finest granularity the 8-bank budget allows.
  * softmax denominator: the V tiles carry a ones column so the
    denominator rides in PSUM row 64 of the P@V output; per (head, slice)
    it is staged to SBUF, DMA-reshaped [1,512]->[64,8] (DVE reciprocal
    cost is per-column, so this is ~20x cheaper than reciprocal on the
    row), reciprocal'd in fp32r, DMA'd back, broadcast across partitions
    by a K=1 fp32r ones-matmul at tile_position (64,0), and applied by a
    DVE multiply (deferred one head-pair so the chain never stalls PE).
  * output returned in bf16 (host upcasts); tail out-projections
    alternate PSUM pools and spread their y DMAs over the gpsimd/sync/
    scalar queues so the drain isn't serialized on one queue.
"""

import sys

if "/opt/trn_rl_repo" not in sys.path:
    sys.path.insert(0, "/opt/trn_rl_repo")

import math

import numpy as np

B, C, E, H, D = 4, 2048, 1024, 16, 64
NCORES = 8
P = 128
CS = 512  # q-slice width


def build_module(C=C, E=E, HL=H // 2, D=D, n_devices=NCORES):
    """Build the SPMD Bass module for one core's shard."""
    from contextlib import ExitStack

    import concourse.bass as bass
    import concourse.mybir as mybir
    import concourse.tile as tile

    F32 = mybir.dt.float32
    FR = mybir.dt.float32r
    BF = mybir.dt.bfloat16
    Exp = mybir.ActivationFunctionType.Exp
    MUL = mybir.AluOpType.mult
    DIV = mybir.AluOpType.divide
    ISGE = mybir.AluOpType.is_ge

    ET = E // P          # e-tiles
    JT = HL * D // P     # j-tiles (head pairs)
    NJ = C // CS         # q-slices
    CT = C // P          # c-tiles
    KPJ = CS // P        # kk-tiles per q-slice (4)
    scale = 1.0 / math.sqrt(D)

    nc = bass.Bass(
        "TRN2", target_bir_lowering=False, debug=False, num_devices=n_devices
    )

    xT = nc.dram_tensor("xT", [P, ET, C], BF, kind="ExternalInput").ap()
    wq_d = nc.dram_tensor("wq", [P, ET, HL * D], BF, kind="ExternalInput").ap()
    wk_d = nc.dram_tensor("wk", [P, ET, HL * D], BF, kind="ExternalInput").ap()
    wv_d = nc.dram_tensor("wv", [P, ET, HL * D], BF, kind="ExternalInput").ap()
    wo_d = nc.dram_tensor("wo", [P, JT, E], BF, kind="ExternalInput").ap()
    y_d = nc.dram_tensor("y", [CT, P, E], BF, kind="ExternalOutput").ap()

    with tile.TileContext(nc) as tc:
        with ExitStack() as ctx:
            pA = ctx.enter_context(tc.tile_pool(name="pA", bufs=1))
            psS = ctx.enter_context(tc.tile_pool(name="psS", bufs=4, space="PSUM"))
            psPV = ctx.enter_context(tc.tile_pool(name="psPV", bufs=2, space="PSUM"))
            psMM = ctx.enter_context(tc.tile_pool(name="psMM", bufs=2, space="PSUM"))
            pX = ctx.enter_context(tc.tile_pool(name="pX", bufs=2))
            pE = ctx.enter_context(tc.tile_pool(name="pE", bufs=10))
            pT = ctx.enter_context(tc.tile_pool(name="pT", bufs=4))
            pH = ctx.enter_context(tc.tile_pool(name="pH", bufs=10))
            pD = ctx.enter_context(tc.tile_pool(name="pD", bufs=2))
            pF = ctx.enter_context(tc.tile_pool(name="pF", bufs=2))
            pR = ctx.enter_context(tc.tile_pool(name="pR", bufs=2))

            qt = pA.tile([P, JT, C], BF, tag="qt")
            kt = pA.tile([P, JT, C], BF, tag="kt")
            v = pA.tile([P, CT, HL, D + 1], BF, tag="v")
            hdt = pA.tile([P, JT, C], BF, tag="hdt")
            ones = pA.tile([P, 64], FR, tag="ones")
            wq = pA.tile([P, ET, HL * D], BF, tag="wq")
            wk = pA.tile([P, ET, HL * D], BF, tag="wk")
            wv = pA.tile([P, ET, HL * D], BF, tag="wv")
            wo = pA.tile([P, JT, E], BF, tag="wo")

            xts = {}

            def load_x(cs, split=False):
                xt = pX.tile([P, ET, CS], BF, tag="xt")
                csl = slice(cs * CS, (cs + 1) * CS)
                if split:
                    h = ET // 2
                    nc.sync.dma_start(xt[:, 0:h, :], xT[:, 0:h, csl])
                    nc.scalar.dma_start(xt[:, h:ET, :], xT[:, h:ET, csl])
                else:
                    nc.sync.dma_start(xt[:], xT[:, :, csl])
                xts[cs] = xt

            # the first matmul chain consumes (wq[et], x0[et]) pairs in
            # order: issue them as alternating per-et transfers on the two
            # HWDGE queues so the chain starts as soon as et=0 lands
            xt0 = pX.tile([P, ET, CS], BF, tag="xt")
            xts[0] = xt0
            # slice-0's et-interleaved Q/K chains consume (wq[et], x0[et],
            # wk[et]) triples in et order: stream them as per-et transfers
            # (per-et completion semaphores) on all three DMA queues
            for et in range(ET):
                qa, qb = (nc.sync, nc.scalar) if et % 2 == 0 else (
                    nc.scalar,
                    nc.sync,
                )
                qa.dma_start(wq[:, et, :], wq_d[:, et, :])
                qb.dma_start(xt0[:, et, :], xT[:, et, 0:CS])
                nc.gpsimd.dma_start(wk[:, et, :], wk_d[:, et, :])
            h = ET // 2
            nc.sync.dma_start(wv[:, 0:h, :], wv_d[:, 0:h, :])
            nc.scalar.dma_start(wv[:, h:ET, :], wv_d[:, h:ET, :])
            nc.gpsimd.dma_start(wo[:], wo_d)
            nc.vector.memset(ones[:].bitcast(F32), 1.0)
            nc.vector.memset(v[:, :, :, D : D + 1], 1.0)

            def proj_units(cs):
                """Projection work for q-slice cs as a list of callables,
                each ~8 matmuls + 1 psum evict."""
                csl = slice(cs * CS, (cs + 1) * CS)

                def qk_unit(w_sb, out_t, jt):
                    def run():
                        xt = xts[cs]
                        ps = psMM.tile([P, CS], F32, tag="mm")
                        for et in range(ET):
                            nc.tensor.matmul(
                                ps[:],
                                w_sb[:, et, jt * P : (jt + 1) * P],
                                xt[:, et, :],
                                start=(et == 0),
                                stop=(et == ET - 1),
                            )
                        nc.vector.tensor_copy(out_t[:, jt, csl], ps[:])

                    return run

                def v_unit(c4):
                    def run():
                        xt = xts[cs]
                        ct = cs * KPJ + c4
                        ps = psMM.tile([P, HL, D], F32, tag="mm")
                        for et in range(ET):
                            nc.tensor.matmul(
                                ps[:],
                                xt[:, et, c4 * P : (c4 + 1) * P],
                                wv[:, et, :],
                                start=(et == 0),
                                stop=(et == ET - 1),
                            )
                        nc.vector.tensor_copy(v[:, ct, :, 0:D], ps[:])

                    return run

                units = []
                if cs == 0:
                    # slice 0 runs at t=0 and is paced by the streaming
                    # wq/x0/wk DMAs: run the 8 Q/K chains et-OUTER across
                    # all 8 PSUM banks (nothing else uses PSUM yet), so
                    # each et's arrival feeds 8 matmuls instead of one and
                    # the PE keeps pace with the DMA queues
                    def qk_interleaved():
                        plan = [(wq, qt, jt) for jt in range(JT)] + [
                            (wk, kt, jt) for jt in range(JT)
                        ]
                        pools = [psMM, psMM, psPV, psPV, psS, psS, psS, psS]
                        tags = {id(psMM): "mm", id(psS): "s", id(psPV): "pv"}
                        ps0 = [
                            pool.tile(
                                [P, CS], F32, tag=tags[id(pool)], name=f"p0{i}"
                            )
                            for i, pool in enumerate(pools)
                        ]
                        for et in range(ET):
                            for i, (w_sb, _, jt) in enumerate(plan):
                                nc.tensor.matmul(
                                    ps0[i][:],
                                    w_sb[:, et, jt * P : (jt + 1) * P],
                                    xts[0][:, et, :],
                                    start=(et == 0),
                                    stop=(et == ET - 1),
                                )
                        for i, (_, out_t, jt) in enumerate(plan):
                            nc.vector.tensor_copy(
                                out_t[:, jt, 0:CS], ps0[i][:]
                            )

                    units.append(qk_interleaved)
                else:
                    for jt in range(JT):
                        units.append(qk_unit(wq, qt, jt))
                        units.append(qk_unit(wk, kt, jt))
                for c4 in range(KPJ):
                    units.append(v_unit(c4))
                return units

            def outproj_units(jj, alt=False):
                """Output projection for the c-tiles of q-slice jj.  With
                alt=True (tail drain, attention done) alternate psMM/psS
                so the PE doesn't wait on single-pool PSUM rotation."""
                FS = min(CS, E)
                units = []
                for c4 in range(KPJ):
                    for fs in range(E // FS):

                        def run(
                            ct=jj * KPJ + c4,
                            fsl=slice(fs * FS, (fs + 1) * FS),
                            pool=(psS if alt and (c4 % 2) else psMM),
                            dq=(
                                [nc.gpsimd, nc.sync, nc.scalar][
                                    (c4 * (E // FS) + fs) % 3
                                ]
                                if alt
                                else nc.gpsimd
                            ),
                        ):
                            ps = pool.tile([P, FS], F32, tag="mm" if pool is psMM else "s")
                            for jt in range(JT):
                                nc.tensor.matmul(
                                    ps[:],
                                    hdt[:, jt, ct * P : (ct + 1) * P],
                                    wo[:, jt, fsl],
                                    start=(jt == 0),
                                    stop=(jt == JT - 1),
                                )
                            ysb = pT.tile([P, FS], BF, tag="ysb")
                            nc.vector.tensor_copy(ysb[:], ps[:])
                            dq.dma_start(y_d[ct, :, fsl], ysb[:])

                        units.append(run)
                return units

            for u in proj_units(0):
                u()

            for j in range(NJ):
                jsl = slice(j * CS, (j + 1) * CS)
                nkt = (j + 1) * KPJ  # kk-tiles needed (causal)
                fillers = []
                if j + 1 < NJ:
                    load_x(j + 1)
                    fillers += proj_units(j + 1)
                # out-projections run one slice later than they become
                # ready (the last normalize of slice jj lands while slice
                # jj+1's first groups run, so jj's outproj can only be
                # dispatched from jj+2); spread them 2/1 so slice 3's
                # DVE/DMA load doesn't spike
                if j == 2:
                    fillers += outproj_units(0) + outproj_units(1)[:4]
                elif j == 3:
                    fillers += outproj_units(1)[4:] + outproj_units(2)
                ngrps = JT * (j + 1)
                gdone = fi = 0
                norms = []

                def emit_norm(g, dt, hds, jsl=jsl):
                    bts = {}

                    def mk_bcast(half):
                        def run():
                            bct = psS.tile([P, CS], F32, tag="s", name="bc")
                            bc = bct[0:64, :]
                            nc.tensor.matmul(
                                bc,
                                ones[64:65, :],
                                dt[D : D + 1, half, :],
                                start=True,
                                stop=True,
                                tile_position=(64, 0),
                            )
                            bts[half] = bc

                        return run

                    def mk_mul(half):
                        def run():
                            hd = hds[half]
                            if half == 0:
                                nc.vector.tensor_tensor(
                                    hdt[0:64, g, jsl], hd[:], bts[half], MUL
                                )
                            else:
                                tmp = pT.tile([64, CS], BF, tag="tmp")
                                nc.vector.tensor_tensor(
                                    tmp[:], hd[:], bts[half], MUL
                                )
                                nc.sync.dma_start(
                                    hdt[64:128, g, jsl], tmp[:]
                                )

                        return run

                    for half in (1, 0):
                        norms.append(mk_bcast(half))
                    for half in (1, 0):
                        norms.append(mk_mul(half))

                pending = None
                for g in range(JT):
                    pv_ps = [
                        psPV.tile([D + 1, CS], F32, tag="pv", name=f"pv{h}")
                        for h in range(2)
                    ]
                    # kk-tiles in groups of 4 (two 2-kt psum chunks) so the
                    # S^T matmuls and the PV accumulation run as longer
                    # back-to-back chains on the PE
                    for grp in range((nkt + 3) // 4):
                        group = []  # (kts, ws, s_ps, e_sb) per 2-kt chunk
                        for ck in (2 * grp, 2 * grp + 1):
                            kts = [k for k in (2 * ck, 2 * ck + 1) if k < nkt]
                            if not kts:
                                continue
                            ws = [max(0, k * P - j * CS) for k in kts]
                            # one PSUM bank per (kt, half): exp releases
                            # banks at kt granularity, so the next chunk's
                            # S matmuls start half a chunk earlier
                            s_ps = [
                                [
                                    psS.tile(
                                        [P, CS], F32, tag="s", name=f"s{h}{i}"
                                    )
                                    for h in range(2)
                                ]
                                for i in range(len(kts))
                            ]
                            e_sb = [
                                pE.tile([P, 2, CS], BF, tag="e", name=f"e{h}")
                                for h in range(2)
                            ]
                            group.append((kts, ws, s_ps, e_sb))
                            for i, kkt in enumerate(kts):
                                w = ws[i]
                                ksl = slice(kkt * P, (kkt + 1) * P)
                                qsl = slice(j * CS + w, (j + 1) * CS)
                                for half, base in ((0, 0), (1, 64)):
                                    nc.tensor.matmul(
                                        s_ps[i][half][:, w:CS],
                                        kt[base : base + 64, g, ksl],
                                        qt[base : base + 64, g, qsl],
                                        start=True,
                                        stop=True,
                                        tile_position=(base, 0),
                                    )
                        for kts, ws, s_ps, e_sb in group:
                            for i in range(len(kts)):
                                w = ws[i]
                                for half in range(2):
                                    nc.scalar.activation(
                                        e_sb[half][:, i, w:CS],
                                        s_ps[i][half][:, w:CS],
                                        Exp,
                                        scale=scale,
                                    )
                            for i, kkt in enumerate(kts):
                                w = ws[i]
                                if 0 <= kkt * P - j * CS < CS:
                                    for half in range(2):
                                        blk = e_sb[half][:, i, w : w + P]
                                        nc.gpsimd.affine_select(
                                            blk,
                                            blk,
                                            pattern=[[1, P]],
                                            compare_op=ISGE,
                                            fill=0.0,
                                            base=0,
                                            channel_multiplier=-1,
                                        )
                        gdone += 1
                        # filler PE work goes BETWEEN this group's S matmuls
                        # and its P@V: the PE queue is in-order, so this is
                        # what actually covers the exp(ACT) latency the P@V
                        # matmuls wait on.  (hold a few units back for the
                        # tail normalize.)
                        want = len(fillers) * gdone // ngrps
                        if grp == (nkt + 3) // 4 - 1:
                            want = max(want, fi + 2)
                        want = min(want, max(0, len(fillers) - 3))
                        while fi < want:
                            fillers[fi]()
                            fi += 1
                        for half in range(2):
                            h = 2 * g + half
                            for kts, ws, s_ps, e_sb in group:
                                for i, kkt in enumerate(kts):
                                    w = ws[i]
                                    nc.tensor.matmul(
                                        pv_ps[half][:, w:CS],
                                        v[:, kkt, h, :],
                                        e_sb[half][:, i, w:CS],
                                        start=(kkt == 0),
                                        stop=(kkt == nkt - 1),
                                    )

                    # evict PV (bf16); the raw denominators ride in PSUM row
                    # 64 (ones column of V) — reciprocal each in place on
                    # that single row (fast approx, ~18 bits), then gpsimd
                    # broadcasts it across partitions for the DVE multiply.
                    hds = []
                    for half in range(2):
                        hd = pH.tile([D, CS], BF, tag="hd", name=f"hd{half}")
                        nc.vector.tensor_copy(hd[:], pv_ps[half][0:D, :])
                        hds.append(hd)
                    # DVE reciprocal cost is per-column, so reshape the
                    # [1,512] denominator row into [64,8] via DMA, take the
                    # reciprocal over 8 columns (~30x cheaper than on the
                    # row), and DMA back for the fp32r broadcast matmul
                    st = pD.tile([D + 1, 2, CS], F32, tag="st")
                    dt = pF.tile([D + 1, 2, CS], FR, tag="dt")
                    dr = pR.tile([64, 2, 8], FR, tag="dr")
                    for half in range(2):
                        nc.vector.tensor_copy(
                            st[D : D + 1, half, :],
                            pv_ps[half][D : D + 1, :],
                        )
                        nc.sync.dma_start(
                            dr[:, half, :].bitcast(F32),
                            st[D : D + 1, half, :],
                        )
                    with nc.allow_low_precision(
                        reason="fp32r reciprocal feeds fp32r matmul"
                    ):
                        for half in range(2):
                            nc.vector.reciprocal(
                                dr[:, half, :], dr[:, half, :]
                            )
                    for half in range(2):
                        nc.sync.dma_start(
                            dt[D : D + 1, half, :],
                            dr[:, half, :],
                        )
                    # previous g's broadcast+normalize first: its psS slot
                    # is exp-released and its DVE multiplies must not queue
                    # behind this g's hd evictions
                    if pending is not None:
                        for u in pending:
                            u()
                    emit_norm(g, dt, hds)
                    pending, norms = norms, []

                while fi < len(fillers):
                    fillers[fi]()
                    fi += 1
                for u in pending:
                    u()

            for u in outproj_units(NJ - 1, alt=True):
                u()
    return nc


def _merge_ldweights_json(bir_json_bytes):
    """tile_legalize pre-splits every non-f32 matmul into a standalone
    InstLdweights + non-self-loading InstMatmult, but walrus's
    --enable-ldw-opt=true rejects pre-split Ldweights ("not compatible
    with LDW optimization").  Merge each pair back into a self-loading
    Matmult (ldweights=True, waits unioned) so walrus can apply its own
    LDWEIGHTS overlap optimization."""
    import json

    d = json.loads(bir_json_bytes)
    n = 0
    for fn in d["functions"]:
        for blk in fn["blocks"]:
            insts = blk["instructions"]
            out = []
            pending = None  # pending Ldweights awaiting its Matmult
            for inst in insts:
                if inst.get("opcode") == "Ldweights":
                    assert pending is None, "two Ldweights without a Matmult"
                    pending = inst
                    continue
                if pending is not None and inst.get("engine") == "PE":
                    assert inst.get("opcode") == "Matmult", (
                        f"Ldweights followed by PE {inst.get('opcode')}"
                    )
                    w_ld = pending["ins"][0]
                    w_mm = inst["ins"][1]
                    assert (
                        w_ld["memref"] == w_mm["memref"]
                        and w_ld["offset"] == w_mm["offset"]
                    ), "Ldweights/Matmult weight AP mismatch"
                    inst["ldweights"] = True
                    lsi = pending.get("sync_info") or {}
                    msi = inst.get("sync_info") or {}
                    waits = (lsi.get("on_wait") or []) + (msi.get("on_wait") or [])
                    updates = (lsi.get("on_update") or []) + (
                        msi.get("on_update") or []
                    )
                    if waits or updates:
                        inst["sync_info"] = {"on_wait": waits, "on_update": updates}
                    pending = None
                    n += 1
                out.append(inst)
            assert pending is None, "trailing Ldweights without a Matmult"
            blk["instructions"] = out
    return json.dumps(d).encode()


def _patch_ldw_opt():
    """Flip walrus to --enable-ldw-opt=true (the merged self-loading
    matmuls above are compatible; the default pre-split ones are not)."""
    import concourse.bass_utils as bu

    if getattr(bu, "_ldw_opt_patched", False):
        return
    bu._ldw_opt_patched = True
    orig = bu.run_command

    def patched(argv, **kw):
        argv = [
            a.replace("--enable-ldw-opt=false", "--enable-ldw-opt=true")
            for a in argv
        ]
        return orig(argv, **kw)

    bu.run_command = patched


def _split_waits_json(bir_json_bytes):
    """TRN2 TPB instructions have one sync-wait slot and this walrus build
    refuses to split multi-wait instructions, so hoist all but the last wait
    onto preceding wait-only EventSemaphore instructions (same engine,
    executed in order -> semantically identical)."""
    import json

    d = json.loads(bir_json_bytes)
    n = 0
    for fn in d["functions"]:
        for blk in fn["blocks"]:
            out = []
            for inst in blk["instructions"]:
                si = inst.get("sync_info")
                waits = (si or {}).get("on_wait") or []
                if len(waits) > 1:
                    for w in waits[:-1]:
                        n += 1
                        out.append(
                            {
                                "debug": inst.get("debug", 0),
                                "engine": inst["engine"],
                                "ins": [],
                                "name": f"wsplit-{n}",
                                "opcode": "EventSemaphore",
                                "outs": [],
                                "sync_info": {"on_update": [], "on_wait": [w]},
                            }
                        )
                    si["on_wait"] = [waits[-1]]
                out.append(inst)
            blk["instructions"] = out
    return json.dumps(d).encode()


def _striped(a, p=P):
    """[K, N] with K = kt*p + i  ->  contiguous [p, K//p, N]."""
    k, n = a.shape
    return np.ascontiguousarray(a.reshape(k // p, p, n).transpose(1, 0, 2))


def _bf16(a):
    import ml_dtypes

    return a.astype(ml_dtypes.bfloat16)


def prep_core_inputs(x_b, wq_s, wk_s, wv_s, wo_s):
    """Host-side layout prep for one core. x_b [C,E], w*_s column/row slices."""
    return {
        "xT": _bf16(_striped(np.ascontiguousarray(x_b.T))),
        "wq": _bf16(_striped(wq_s)),
        "wk": _bf16(_striped(wk_s)),
        "wv": _bf16(_striped(wv_s)),
        "wo": _bf16(_striped(wo_s)),
    }


_module_cache = {}


def kernel(x, W_q, W_k, W_v, W_o):
    from concourse.bass_utils import run_bass_kernel_spmd

    x = np.asarray(x, dtype=np.float32)
    W_q = np.asarray(W_q, dtype=np.float32)
    W_k = np.asarray(W_k, dtype=np.float32)
    W_v = np.asarray(W_v, dtype=np.float32)
    W_o = np.asarray(W_o, dtype=np.float32)

    HD2 = H * D // 2  # columns per head-group (512)
    in_maps = []
    for core in range(NCORES):
        b, hg = core // 2, core % 2
        cols = slice(hg * HD2, (hg + 1) * HD2)
        in_maps.append(
            prep_core_inputs(
                x[b], W_q[:, cols], W_k[:, cols], W_v[:, cols], W_o[cols, :]
            )
        )

    if "nc" not in _module_cache:
        _patch_ldw_opt()
        nc = build_module()
        fixed = _split_waits_json(_merge_ldweights_json(nc.to_json_bytes()))
        nc.to_json_bytes = lambda: fixed
        _module_cache["nc"] = nc
    nc = _module_cache["nc"]

    res = run_bass_kernel_spmd(nc, in_maps, core_ids=list(range(NCORES)))
    _module_cache["last_res"] = res
    out = np.empty((B, C, E), dtype=np.float32)
    for b in range(B):
        ya = res.results[2 * b]["y"].astype(np.float32).reshape(C, E)
        yb = res.results[2 * b + 1]["y"].astype(np.float32).reshape(C, E)
        out[b] = ya + yb
    return out


if __name__ == "__main__":
    rng = np.random.default_rng(0)
    ins = {
        "x": rng.standard_normal((B, C, E), dtype=np.float32),
        "W_q": rng.standard_normal((E, H * D), dtype=np.float32) * 0.02,
        "W_k": rng.standard_normal((E, H * D), dtype=np.float32) * 0.02,
        "W_v": rng.standard_normal((E, H * D), dtype=np.float32) * 0.02,
        "W_o": rng.standard_normal((H * D, E), dtype=np.float32) * 0.02,
    }
    out = kernel(**ins)
    print("kernel ran, out shape", out.shape, "mean", out.mean())

